# revision 45
# baseline (speedup 1.0000x reference)
"""Trainium2 Bass kernel for single-head cross-attention with additive mask.

Computation (matches the reference):
    q = tgt @ wq + bq
    k = src @ wk (+ bk dropped: softmax cancels a per-row constant exactly)
    v = src @ wv (bv folded into the epilogue: out = attn@v + bv)
    s = (q k^T + mask) / sqrt(DQ)
    out = softmax(s) @ v + bv

Two SPMD launches on 8 cores:
  L1: each core projects kT (fp32 psum) and v (fp16 matmuls) for 1/8 of the
      global (B*S) src rows from a host-pre-transposed src slice.
  host: concatenates the 8 K/V shards, appends the softmax-denominator ones
      column to V, casts K/V to fp16 (pure layout glue, no math).
  L2: tgt sharded 8 ways; core c handles tgt rows [c*512,(c+1)*512) of every
      batch so its 8MB mask slice is read from HBM exactly once.

Scores are built transposed (src rows on PSUM partitions) so the PV matmul
consumes softmax weights directly, batch-pair outer so the QK psum tile can
triple-buffer. Projections accumulate in fp32; Q/K/V/mask/P are fp16 (11
mantissa bits keeps rel-err ~1e-3 at full matmul speed). The mask is added
by the otherwise-idle DVE (fp16-cast on the fly by the load DMA), exp() runs
on ACT and emits fp16 attention weights, PV accumulates fp32 in PSUM, and
the epilogue (1/rowsum scaling + bv bias) is PE-free via gpsimd
partition_broadcast. The output leaves transposed [B, DQ, TS]; the host
flips it.
"""
import numpy as np

B, S, D, DQ = 4, 4096, 1024, 64
NCORES = 8
TS = S // NCORES            # 512 tgt rows per core
SR = (B * S) // NCORES      # 2048 global src rows per core (L1)
SB = S // 128               # 32 src blocks per batch
GK = B * SB                 # 128 global src blocks
CORES = list(range(NCORES))
F32 = np.float32
FP16 = np.float16

_CACHE = {}


def _build_l1():
    import concourse.mybir as mybir
    import concourse.tile as tile
    from concourse import bacc

    f32 = mybir.dt.float32
    fp16 = mybir.dt.float16

    nc = bacc.Bacc("TRN2", target_bir_lowering=False, debug=False,
                   num_devices=NCORES)
    srcT = nc.dram_tensor("srcT", [D, SR], f32, kind="ExternalInput")
    wk = nc.dram_tensor("wk", [D, DQ], f32, kind="ExternalInput")
    wv = nc.dram_tensor("wv", [D, DQ], f32, kind="ExternalInput")
    kt = nc.dram_tensor("kt", [DQ, 2, 1024], f32, kind="ExternalOutput")
    vout = nc.dram_tensor("vout", [SR, DQ], f32, kind="ExternalOutput")

    with tile.TileContext(nc) as tc:
        with (
            tc.tile_pool(name="const", bufs=1) as constp,
            tc.tile_pool(name="big", bufs=1) as bigp,
            tc.tile_pool(name="stream", bufs=2) as streamp,
            tc.tile_pool(name="pp", bufs=1, space="PSUM") as pp,
        ):
            wk_sb = constp.tile([128, 8 * DQ], f32)
            nc.sync.dma_start(
                out=wk_sb.rearrange("p (j m) -> p j m", m=DQ),
                in_=wk.rearrange("(j p) m -> p j m", p=128))
            wv_bf = constp.tile([128, 8 * DQ], fp16)
            nc.gpsimd.dma_start(
                out=wv_bf.rearrange("p (j m) -> p j m", m=DQ),
                in_=wv.rearrange("(j p) m -> p j m", p=128))

            kT_psA = pp.tile([128, 1024], f32, tag="qk0")
            kT_psB = pp.tile([128, 1024], f32, tag="qk1")
            v_ps = [pp.tile([128, 4 * DQ], f32, tag=f"pv{q}", name=f"v_ps{q}")
                    for q in range(4)]
            for j in range(8):
                st = streamp.tile([128, SR], f32, tag="xs", bufs=3)
                nc.sync.dma_start(out=st[:], in_=srcT[j * 128:(j + 1) * 128, :])
                stb = streamp.tile([128, SR], fp16, tag="xsb")
                nc.vector.tensor_copy(stb[:], st[:])
                for g in (0, 2, 1, 3):  # alternate col-groups for PE overlap
                    if g < 2:
                        ps, col, tp, po = kT_psA, g * 512, (0, 0), 0
                    else:
                        ps, col, tp, po = kT_psB, (g - 2) * 512, (0, 64), 64
                    nc.tensor.matmul(
                        ps[po:po + 64, col:col + 512],
                        lhsT=wk_sb[:, j * DQ:(j + 1) * DQ],
                        rhs=st[:, g * 512:(g + 1) * 512],
                        start=(j == 0), stop=(j == 7), tile_position=tp)
                for k in range(16):
                    nc.tensor.matmul(
                        v_ps[k // 4][:, (k % 4) * DQ:(k % 4 + 1) * DQ],
                        lhsT=stb[:, k * 128:(k + 1) * 128],
                        rhs=wv_bf[:, j * DQ:(j + 1) * DQ],
                        start=(j == 0 and k % 4 == 0),
                        stop=(j == 7 and k % 4 == 3))
            kT_sb = bigp.tile([128, 1024], f32)
            nc.scalar.copy(kT_sb[0:64, :], kT_psA[0:64, :])
            nc.scalar.copy(kT_sb[64:128, :], kT_psB[64:128, :])
            v_sb = bigp.tile([128, 16 * DQ], f32)
            for q in range(4):
                nc.vector.tensor_copy(v_sb[:, q * 256:(q + 1) * 256], v_ps[q][:])
            nc.sync.dma_start(out=kt[:, 0, :], in_=kT_sb[0:64, :])
            nc.sync.dma_start(out=kt[:, 1, :], in_=kT_sb[64:128, :])
            nc.gpsimd.dma_start(
                out=vout.rearrange("(k p) d -> p k d", p=128),
                in_=v_sb.rearrange("p (k d) -> p k d", d=DQ))
    nc.compile()
    return nc


def _build_l2():
    import concourse.mybir as mybir
    import concourse.tile as tile
    from concourse import bacc
    from concourse.masks import make_identity

    f32 = mybir.dt.float32
    fp16 = mybir.dt.float16
    AF = mybir.ActivationFunctionType

    nc = bacc.Bacc("TRN2", target_bir_lowering=False, debug=False,
                   num_devices=NCORES)
    # kT2 layout: partitions 0-63 = d, s of batches 0-1; 64-127 = batches 2-3
    kt2d = nc.dram_tensor("kt2", [128, 2 * S], fp16, kind="ExternalInput")
    # v65 in SBUF layout: row p, cols (k, c): element = v[k*128 + p, c] | ones
    v65d = nc.dram_tensor("v65", [128, GK * (DQ + 1)], fp16, kind="ExternalInput")
    tgtT = nc.dram_tensor("tgtT", [B, D, TS], f32, kind="ExternalInput")
    # host-transposed mask slice: masknT[s, t] = mask[c*TS + t, s]
    masknT = nc.dram_tensor("masknT", [S, TS], f32, kind="ExternalInput")
    wq = nc.dram_tensor("wq", [D, DQ], f32, kind="ExternalInput")
    bq = nc.dram_tensor("bq", [DQ], f32, kind="ExternalInput")
    bv = nc.dram_tensor("bv", [DQ], f32, kind="ExternalInput")
    # transposed output: host flips [B, DQ, TS] -> [B, TS, DQ]
    out = nc.dram_tensor("out", [B, DQ, TS], f32, kind="ExternalOutput")

    with tile.TileContext(nc) as tc:
        with (
            tc.tile_pool(name="const", bufs=1) as constp,
            tc.tile_pool(name="big", bufs=1) as bigp,
            tc.tile_pool(name="stream", bufs=2) as streamp,
            tc.tile_pool(name="pp", bufs=1, space="PSUM") as pp,
        ):
            wq_sb = constp.tile([128, 8 * DQ], f32)
            nc.sync.dma_start(
                out=wq_sb.rearrange("p (j m) -> p j m", m=DQ),
                in_=wq.rearrange("(j p) m -> p j m", p=128))
            bq_sb = constp.tile([128, 1], f32)
            nc.sync.dma_start(out=bq_sb[0:64, :], in_=bq.rearrange("(p o) -> p o", o=1))
            nc.sync.dma_start(out=bq_sb[64:128, :], in_=bq.rearrange("(p o) -> p o", o=1))
            bv_sb = constp.tile([64, 1], f32)
            nc.sync.dma_start(out=bv_sb[:], in_=bv.rearrange("(p o) -> p o", o=1))

            # resident loads, chunked so sg=0 unblocks early
            kT2 = bigp.tile([128, 2 * S], fp16)
            for q4 in (0, 2, 1, 3):  # first halves of both batch-halves first
                nc.sync.dma_start(out=kT2[:, q4 * 2048:(q4 + 1) * 2048],
                                  in_=kt2d[:, q4 * 2048:(q4 + 1) * 2048])
            v2 = bigp.tile([128, GK * (DQ + 1)], fp16)
            VQ = 32 * (DQ + 1)
            for q4 in range(4):
                nc.gpsimd.dma_start(out=v2[:, q4 * VQ:(q4 + 1) * VQ],
                                    in_=v65d[:, q4 * VQ:(q4 + 1) * VQ])
            # maskT, fp16-cast on the fly, duplicated per batch-half so one
            # [128, 1024] DVE add covers a whole score-pair tile:
            # layout [128 s-partitions, (sg, half, t)]
            maskTd = bigp.tile([128, SB * 2 * TS], fp16)
            mview = maskTd.rearrange("p (sb h t) -> p sb h t", h=2, t=TS)
            for g in range(4):
                nc.gpsimd.dma_start(
                    out=mview[:, g * 8:(g + 1) * 8, 0, :],
                    in_=masknT[g * 1024:(g + 1) * 1024, :]
                    .rearrange("(sb p) t -> p sb t", p=128))
                nc.vector.tensor_copy(mview[:, g * 8:(g + 1) * 8, 1, :],
                                      mview[:, g * 8:(g + 1) * 8, 0, :])

            # qT projection (fp32 matmuls, fp16 output for the fp16 QK)
            qT_sb = bigp.tile([128, 2 * TS], fp16)
            for b in range(B):
                pb, colb = (b // 2) * 64, (b % 2) * TS
                q_ps = pp.tile([128, TS], f32, tag="qk", bufs=3, name=f"q_ps{b}")
                for half in range(2):
                    tg = streamp.tile([128, SR], f32, tag="xs", bufs=3,
                                      name=f"tg{b}_{half}")
                    nc.sync.dma_start(
                        out=tg.rearrange("p (j t) -> p j t", t=TS),
                        in_=tgtT[b, half * 512:(half + 1) * 512, :]
                        .rearrange("(j p) t -> p j t", p=128))
                    for jj in range(4):
                        j = half * 4 + jj
                        nc.tensor.matmul(
                            q_ps[pb:pb + 64, :],
                            lhsT=wq_sb[:, j * DQ:(j + 1) * DQ],
                            rhs=tg[:, jj * TS:(jj + 1) * TS],
                            start=(j == 0), stop=(j == 7), tile_position=(0, pb))
                nc.scalar.activation(
                    qT_sb[pb:pb + 64, colb:colb + TS], q_ps[pb:pb + 64, :],
                    AF.Identity, bias=bq_sb[pb:pb + 64, :])

            # attention main loop: batch-pair outer so the QK psum tile can
            # triple-buffer (3 x 2 banks) against the DVE/ACT consumers.
            for pair in range(2):
                pb = pair * 64
                pv_ps = [pp.tile([65, TS], f32, tag=f"pv{h}",
                                 name=f"pv_ps{pair}_{h}") for h in range(2)]
                for sg in range(SB):
                    qkt = pp.tile([128, 2 * TS], f32, tag="qk", bufs=3,
                                  name=f"qkt{pair}_{sg}")
                    for half in range(2):
                        nc.tensor.matmul(
                            qkt[:, half * TS:(half + 1) * TS],
                            lhsT=kT2[pb:pb + 64, half * S + sg * 128:
                                     half * S + sg * 128 + 128],
                            rhs=qT_sb[pb:pb + 64, half * TS:(half + 1) * TS],
                            start=True, stop=True, tile_position=(pb, 0))
                    es = streamp.tile([128, 2 * TS], f32, tag="E", bufs=4,
                                      name=f"es{pair}_{sg}")
                    nc.vector.tensor_add(
                        es[:], qkt[:],
                        maskTd[:, sg * 2 * TS:(sg + 1) * 2 * TS])
                    pt = streamp.tile([128, 2 * TS], fp16, tag="P", bufs=6,
                                      name=f"pt{pair}_{sg}")
                    nc.scalar.activation(pt[:], es[:], AF.Exp, scale=0.125)
                    for half in range(2):
                        b = pair * 2 + half
                        kg = b * SB + sg
                        nc.tensor.matmul(
                            pv_ps[half][:],
                            lhsT=v2[:, kg * (DQ + 1):(kg + 1) * (DQ + 1)],
                            rhs=pt[:, half * TS:(half + 1) * TS],
                            start=(sg == 0), stop=(sg == SB - 1))

                # epilogue: out^T = pv[0:64]/sums + bv, all PE-free
                for half in range(2):
                    b = pair * 2 + half
                    sums = streamp.tile([65, TS], f32, tag="sums")
                    nc.scalar.copy(sums[64:65, :], pv_ps[half][64:65, :])
                    sums0 = streamp.tile([1, TS], f32, tag="sums0")
                    nc.sync.dma_start(out=sums0[:], in_=sums[64:65, :])
                    recip = streamp.tile([1, TS], f32, tag="recip")
                    nc.vector.reciprocal(recip[:], sums0[:])
                    rb = streamp.tile([64, TS], f32, tag="rb")
                    nc.gpsimd.partition_broadcast(rb[:], recip[:])
                    ot = streamp.tile([64, TS], f32, tag="ot")
                    nc.vector.tensor_mul(ot[:], pv_ps[half][0:64, :], rb[:])
                    of = streamp.tile([64, TS], f32, tag="of")
                    nc.scalar.activation(of[:], ot[:], AF.Identity, bias=bv_sb[:])
                    nc.gpsimd.dma_start(out=out[b], in_=of[:])
    nc.compile()
    return nc


def _get_l1():
    if "l1" not in _CACHE:
        _CACHE["l1"] = _build_l1()
    return _CACHE["l1"]


def _get_l2():
    if "l2" not in _CACHE:
        _CACHE["l2"] = _build_l2()
    return _CACHE["l2"]


def make_in_maps_l1(src, wk, wv):
    src_flat = np.ascontiguousarray(src, dtype=F32).reshape(B * S, D)
    wk = np.ascontiguousarray(wk, dtype=F32)
    wv = np.ascontiguousarray(wv, dtype=F32)
    return [{
        "srcT": np.ascontiguousarray(src_flat[c * SR:(c + 1) * SR, :].T),
        "wk": wk, "wv": wv,
    } for c in CORES]


def glue_l1_outputs(results):
    """Assemble full kT2 / v65 arrays from the 8 per-core L1 outputs."""
    kts = [np.asarray(results[c]["kt"]).reshape(DQ, 2 * 1024) for c in CORES]
    kT_full = np.concatenate(kts, axis=1)            # [64, B*S]
    kt2 = np.concatenate([kT_full[:, :2 * S], kT_full[:, 2 * S:]],
                         axis=0).astype(FP16)
    v_full = np.concatenate(
        [np.asarray(results[c]["vout"]) for c in CORES], axis=0)  # [B*S, 64]
    v65 = np.empty((B * S, DQ + 1), dtype=FP16)
    v65[:, :DQ] = v_full.astype(FP16)
    v65[:, DQ] = np.asarray(1.0, dtype=FP16)
    # rearrange to the L2 SBUF layout: [128 partitions, (block k, col c)]
    v65 = np.ascontiguousarray(
        v65.reshape(GK, 128, DQ + 1).transpose(1, 0, 2).reshape(128, -1))
    return np.ascontiguousarray(kt2), v65


def make_in_maps_l2(kt2, v65, tgt, mask, wq, bq, bv):
    tgt = np.ascontiguousarray(tgt, dtype=F32)
    mask = np.ascontiguousarray(mask, dtype=F32)
    wq = np.ascontiguousarray(wq, dtype=F32)
    bq = np.ascontiguousarray(bq, dtype=F32)
    bv = np.ascontiguousarray(bv, dtype=F32)
    return [{
        "kt2": kt2, "v65": v65,
        "tgtT": np.ascontiguousarray(
            tgt[:, c * TS:(c + 1) * TS, :].transpose(0, 2, 1)),
        "masknT": np.ascontiguousarray(mask[c * TS:(c + 1) * TS, :].T),
        "wq": wq, "bq": bq, "bv": bv,
    } for c in CORES]


def kernel(src, tgt, mask, wq, bq, wk, bk, wv, bv):
    from concourse.bass_utils import run_bass_kernel_spmd

    res1 = run_bass_kernel_spmd(_get_l1(), make_in_maps_l1(src, wk, wv),
                                core_ids=CORES)
    kt2, v65 = glue_l1_outputs(res1.results)
    res2 = run_bass_kernel_spmd(
        _get_l2(), make_in_maps_l2(kt2, v65, tgt, mask, wq, bq, bv),
        core_ids=CORES)
    out = np.empty((B, S, DQ), dtype=F32)
    for c in CORES:
        out[:, c * TS:(c + 1) * TS, :] = \
            np.asarray(res2.results[c]["out"]).transpose(0, 2, 1)
    return out
